# revision 17
# baseline (speedup 1.0000x reference)
"""
Trainium2 Bass kernel for nn_MetaAttention.

Computation (per batch b):
    rowsum[h,i]     = sum_j m[b,h,i,j]
    aggregated[i,j] = sum_h rowsum[h,i] * m[b,h,i,j]
    out[b]          = softmax(aggregated.flatten()).reshape(N, N)

Sharding: pure data parallel over B=16 across 8 cores (2 batches/core).

Per-core kernel strategy (memory-bound, ~64 MB HBM traffic/core):
  - Row tiles of P=112 partitions (784 = 7*112).
  - Per (batch, chunk of row tiles, head): one big DMA load.
  - rowsum via free-axis reduce (split across Vector and Scalar engines).
  - scale-by-rowsum AND accumulate-over-heads fused into ONE TensorEngine
    matmul: lhsT = diag(rowsum) (built by Vector from a constant identity),
    accumulating into PSUM across the 12 heads.
  - Global softmax: per-tile max (Vector), cross-partition max/sum via
    GpSimd partition_all_reduce, exp with fused accumulated sums on Scalar,
    final scale on Vector, single DMA store per batch.
"""

import numpy as np

B, H, N = 16, 12, 784
NCORES = 8
BPC = B // NCORES          # batches per core
P = 112                    # partition tile (784 = 7 * 112)
NT = N // P                # 7 row tiles
CHUNKS = [(0, 4), (4, 3)]  # (first row-tile, n row-tiles) per PSUM chunk
JSPLITS = [(0, 512), (512, 272)]  # matmul free-dim splits (PSUM bank aligned)
# heads whose rowsum runs on Vector (rest on Scalar)
VEC_ROWSUM_H = {0, 1, 2, 3, 4, 5, 6, 8, 10}

LAST_RESULT = None  # BassKernelResults of the most recent kernel() call


def build_program():
    import concourse.bacc as bacc
    import concourse.tile as tile
    from concourse import mybir

    f32 = mybir.dt.float32
    nc = bacc.Bacc("TRN2")

    x = nc.dram_tensor("x", [BPC, H, N, N], f32, kind="ExternalInput")
    ident = nc.dram_tensor("ident", [P, P], f32, kind="ExternalInput")
    y = nc.dram_tensor("y", [BPC, N, N], f32, kind="ExternalOutput")

    with tile.TileContext(nc) as tc:
        with (
            tc.tile_pool(name="mh", bufs=6) as mh_pool,
            tc.tile_pool(name="agg", bufs=2) as agg_pool,
            tc.tile_pool(name="acc", bufs=4, space="PSUM") as acc_pool,
            tc.tile_pool(name="diag", bufs=4) as diag_pool,
            tc.tile_pool(name="scratch", bufs=3) as scratch_pool,
            tc.tile_pool(name="small", bufs=8) as small_pool,
            tc.tile_pool(name="consts", bufs=1) as const_pool,
        ):
            ident_sb = const_pool.tile([P, P], f32)
            nc.sync.dma_start(out=ident_sb, in_=ident[:, :])
            ones_sb = const_pool.tile([P, P], f32)
            nc.vector.memset(ones_sb, 1.0)

            for b in range(BPC):
                agg = agg_pool.tile([P, NT, N], f32, tag="agg")
                maxs = small_pool.tile([P, NT], f32, tag="maxs")
                sums = small_pool.tile([P, NT], f32, tag="sums")

                for c0, ct in CHUNKS:
                    accs = [
                        acc_pool.tile([P, 1024], f32, tag="acc", name=f"acc_{b}_{c0}_{k}")
                        for k in range(ct)
                    ]
                    for h in range(H):
                        mh = mh_pool.tile([P, ct, N], f32, tag="mh")
                        src = x[b, h].rearrange("(t p) j -> p t j", p=P)
                        nc.sync.dma_start(out=mh, in_=src[:, c0 : c0 + ct, :])
                        for k in range(ct):
                            rs = small_pool.tile([P, 1], f32, tag="rs")
                            if h in VEC_ROWSUM_H:
                                nc.vector.tensor_reduce(
                                    out=rs,
                                    in_=mh[:, k, :],
                                    axis=mybir.AxisListType.X,
                                    op=mybir.AluOpType.add,
                                )
                            else:
                                scr = scratch_pool.tile([P, N], f32, tag="scr")
                                nc.scalar.activation(
                                    out=scr,
                                    in_=mh[:, k, :],
                                    func=mybir.ActivationFunctionType.Copy,
                                    bias=0.0,
                                    scale=1.0,
                                    accum_out=rs,
                                )
                            # diag(rs) = identity * rs, on ScalarE (per-partition
                            # scale); TensorScalar on DVE trips a walrus wait-slot
                            # limit, Activation does not.
                            dg = diag_pool.tile([P, P], f32, tag="dg")
                            nc.scalar.activation(
                                out=dg,
                                in_=ident_sb,
                                func=mybir.ActivationFunctionType.Copy,
                                bias=0.0,
                                scale=rs,
                            )
                            for j0, jn in JSPLITS:
                                nc.tensor.matmul(
                                    accs[k][:, j0 : j0 + jn],
                                    lhsT=dg,
                                    rhs=mh[:, k, j0 : j0 + jn],
                                    start=(h == 0),
                                    stop=(h == H - 1),
                                )
                    # evacuate chunk PSUM -> SBUF, compute per-tile max
                    for k in range(ct):
                        it = c0 + k
                        nc.scalar.copy(out=agg[:, it, :], in_=accs[k][:, 0:N])
                        nc.vector.tensor_reduce(
                            out=maxs[:, it : it + 1],
                            in_=agg[:, it, :],
                            axis=mybir.AxisListType.X,
                            op=mybir.AluOpType.max,
                        )

                # ---- softmax over the full [N, N] of this batch ----
                # cross-partition max: PE transpose -> free-axis reduce ->
                # broadcast back via a K=1 all-ones matmul.
                m1 = small_pool.tile([P, 1], f32, tag="m1")
                nc.vector.tensor_reduce(
                    out=m1, in_=maxs, axis=mybir.AxisListType.X,
                    op=mybir.AluOpType.max,
                )
                tps = acc_pool.tile([1, P], f32, tag="acc", name=f"tps_{b}")
                nc.tensor.transpose(tps, m1, ident_sb)
                gm = small_pool.tile([1, 1], f32, tag="gm")
                nc.vector.tensor_reduce(
                    out=gm, in_=tps, axis=mybir.AxisListType.X,
                    op=mybir.AluOpType.max,
                )
                bps = acc_pool.tile([P, 1], f32, tag="acc", name=f"bps_{b}")
                nc.tensor.matmul(bps, lhsT=ones_sb[0:1, :], rhs=gm,
                                 start=True, stop=True)
                negmax = small_pool.tile([P, 1], f32, tag="negmax")
                nc.scalar.mul(out=negmax, in_=bps, mul=-1.0)

                for it in range(NT):
                    nc.scalar.activation(
                        out=agg[:, it, :],
                        in_=agg[:, it, :],
                        func=mybir.ActivationFunctionType.Exp,
                        bias=negmax,
                        scale=1.0,
                        accum_out=sums[:, it : it + 1],
                    )
                s1 = small_pool.tile([P, 1], f32, tag="s1")
                nc.vector.tensor_reduce(
                    out=s1, in_=sums, axis=mybir.AxisListType.X,
                    op=mybir.AluOpType.add,
                )
                # cross-partition sum + broadcast in one all-ones matmul
                sps = acc_pool.tile([P, 1], f32, tag="acc", name=f"sps_{b}")
                nc.tensor.matmul(sps, lhsT=ones_sb, rhs=s1, start=True, stop=True)
                rinv = small_pool.tile([P, 1], f32, tag="rinv")
                nc.vector.reciprocal(out=rinv, in_=sps)

                for it in range(NT):
                    nc.scalar.activation(
                        out=agg[:, it, :],
                        in_=agg[:, it, :],
                        func=mybir.ActivationFunctionType.Copy,
                        bias=0.0,
                        scale=rinv,
                    )
                dst = y[b].rearrange("(t p) j -> p t j", p=P)
                nc.sync.dma_start(out=dst, in_=agg)

    nc.finalize()  # Bacc: register alloc, nop/event-sem legalization, ISA codegen
    return nc


def kernel(mha_masks) -> np.ndarray:
    global LAST_RESULT
    from concourse.bass_utils import run_bass_kernel_spmd

    xfull = np.ascontiguousarray(np.asarray(mha_masks, dtype=np.float32))
    assert xfull.shape == (B, H, N, N), xfull.shape

    nc = build_program()
    ident = np.eye(P, dtype=np.float32)
    in_maps = [
        {"x": xfull[i * BPC : (i + 1) * BPC], "ident": ident}
        for i in range(NCORES)
    ]
    import os

    kw = {}
    if os.environ.get("KERNEL_TRACE_DIR"):
        kw = dict(trace=True, tmpdir=os.environ["KERNEL_TRACE_DIR"])
    res = run_bass_kernel_spmd(nc, in_maps, core_ids=list(range(NCORES)), **kw)
    LAST_RESULT = res
    out = np.concatenate(
        [np.asarray(r["y"], dtype=np.float32) for r in res.results], axis=0
    )
    return out


# revision 20
# speedup vs baseline: 1.0966x; 1.0966x over previous
"""
Trainium2 Bass kernel for nn_MetaAttention.

Computation (per batch b):
    rowsum[h,i]     = sum_j m[b,h,i,j]
    aggregated[i,j] = sum_h rowsum[h,i] * m[b,h,i,j]
    out[b]          = softmax(aggregated.flatten()).reshape(N, N)

Sharding: pure data parallel over B=16 across 8 cores (2 batches/core).

Per-core kernel strategy (memory regime, ~64 MB HBM traffic/core):
  - Row tiles of P=112 partitions; partition p holds CONTIGUOUS rows
    7p..7p+6 ("(p t) j" mapping) so DMA descriptors are 12-22 KB
    contiguous DRAM segments (near-peak HBM streaming). Row permutation
    is transparent: all math is row-independent, store inverts the map.
  - The scale-and-accumulate over heads is split across three engine
    paths to balance load (fp32 PE matmul is 2-pass + half-rate, so PE
    alone can't carry it):
      * PE path: lhsT=diag(rowsum) matmul accumulating into PSUM.
      * DVE path: fused scalar_tensor_tensor agg = m*rs + agg in SBUF.
      * GPSIMD path: same fused op on the otherwise-idle gpsimd.
    A final DVE add merges the PSUM partial into the SBUF agg.
  - rowsums: DVE tensor_scalar+accum (2x rate) or ACT activation+accum.
  - Global softmax: per-tile max (DVE), cross-partition max/sum via PE
    transpose + all-ones matmul broadcast, exp with fused sums on ACT,
    final scale on DVE, one DMA store per batch.
"""

import numpy as np

B, H, N = 16, 12, 784
NCORES = 8
BPC = B // NCORES          # batches per core
P = 112                    # partition tile (784 = 7 * 112)
NT = N // P                # 7 row tiles
CHUNKS = [(0, 4), (4, 3)]  # (first row-tile, n row-tiles) per PSUM chunk
JSPLITS = [(0, 512), (512, 272)]  # matmul free-dim splits (PSUM bank aligned)

# Per-head path assignment for the scale-accumulate (tuned vs HW profile).
PE_H = {0, 2, 4, 6, 8, 10}   # diag-matmul into PSUM
DVE_H = {1, 9, 11}           # fused m*rs+agg on Vector
GPS_H = {3, 5, 7}            # ACT mult + GPSIMD add (no fused op on Pool)
FIRST_SBUF_H = 1             # first non-PE head (initializes agg via mult)
ROWSUM_DVE_H = {0, 1, 2, 3, 4, 6, 8}  # rowsum on Vector; rest on Scalar

LAST_RESULT = None  # BassKernelResults of the most recent kernel() call


def build_program():
    import concourse.bacc as bacc
    import concourse.tile as tile
    from concourse import mybir

    f32 = mybir.dt.float32
    nc = bacc.Bacc("TRN2")

    x = nc.dram_tensor("x", [BPC, H, N, N], f32, kind="ExternalInput")
    ident = nc.dram_tensor("ident", [P, P], f32, kind="ExternalInput")
    y = nc.dram_tensor("y", [BPC, N, N], f32, kind="ExternalOutput")

    with tile.TileContext(nc) as tc:
        with (
            tc.tile_pool(name="mh", bufs=7) as mh_pool,
            tc.tile_pool(name="agg", bufs=2) as agg_pool,
            tc.tile_pool(name="acc", bufs=4, space="PSUM") as acc_pool,
            tc.tile_pool(name="diag", bufs=4) as diag_pool,
            tc.tile_pool(name="scratch", bufs=4) as scratch_pool,
            tc.tile_pool(name="small", bufs=8) as small_pool,
            tc.tile_pool(name="consts", bufs=1) as const_pool,
        ):
            ident_sb = const_pool.tile([P, P], f32)
            nc.sync.dma_start(out=ident_sb, in_=ident[:, :])
            ones_sb = const_pool.tile([P, P], f32)
            nc.vector.memset(ones_sb, 1.0)

            for b in range(BPC):
                agg = agg_pool.tile([P, NT, N], f32, tag="agg")
                maxs = small_pool.tile([P, NT], f32, tag="maxs")
                sums = small_pool.tile([P, NT], f32, tag="sums")

                for c0, ct in CHUNKS:
                    accs = [
                        acc_pool.tile([P, 1024], f32, tag="acc", name=f"acc_{b}_{c0}_{k}")
                        for k in range(ct)
                    ]
                    for h in range(H):
                        mh = mh_pool.tile([P, ct, N], f32, tag="mh")
                        # partition p <- contiguous rows 7p..7p+6 of m[b,h]
                        src = x[b, h].rearrange("(p t) j -> p t j", p=P)
                        nc.sync.dma_start(out=mh, in_=src[:, c0 : c0 + ct, :])
                        for k in range(ct):
                            it = c0 + k
                            rs = small_pool.tile([P, 1], f32, tag="rs")
                            if h in ROWSUM_DVE_H:
                                scr = scratch_pool.tile([P, N], f32, tag="scr")
                                nc.vector.tensor_scalar(
                                    out=scr,
                                    in0=mh[:, k, :],
                                    scalar1=1.0,
                                    scalar2=None,
                                    op0=mybir.AluOpType.mult,
                                    op1=mybir.AluOpType.add,
                                    accum_out=rs,
                                )
                            else:
                                scr = scratch_pool.tile([P, N], f32, tag="scr")
                                nc.scalar.activation(
                                    out=scr,
                                    in_=mh[:, k, :],
                                    func=mybir.ActivationFunctionType.Copy,
                                    bias=0.0,
                                    scale=1.0,
                                    accum_out=rs,
                                )
                            if h in PE_H:
                                dg = diag_pool.tile([P, P], f32, tag="dg")
                                nc.vector.tensor_scalar_mul(
                                    out=dg, in0=ident_sb, scalar1=rs
                                )
                                first = h == min(PE_H)
                                last = h == max(PE_H)
                                for j0, jn in JSPLITS:
                                    nc.tensor.matmul(
                                        accs[k][:, j0 : j0 + jn],
                                        lhsT=dg,
                                        rhs=mh[:, k, j0 : j0 + jn],
                                        start=first,
                                        stop=last,
                                    )
                            elif h == FIRST_SBUF_H:
                                nc.vector.tensor_scalar_mul(
                                    out=agg[:, it, :], in0=mh[:, k, :], scalar1=rs
                                )
                            elif h in DVE_H:
                                nc.vector.scalar_tensor_tensor(
                                    out=agg[:, it, :],
                                    in0=mh[:, k, :],
                                    scalar=rs,
                                    in1=agg[:, it, :],
                                    op0=mybir.AluOpType.mult,
                                    op1=mybir.AluOpType.add,
                                )
                            else:
                                # gpsimd path: scale on ACT, add on gpsimd
                                sc2 = scratch_pool.tile([P, N], f32, tag="sc2")
                                nc.scalar.activation(
                                    out=sc2,
                                    in_=mh[:, k, :],
                                    func=mybir.ActivationFunctionType.Copy,
                                    bias=0.0,
                                    scale=rs,
                                )
                                nc.gpsimd.tensor_tensor(
                                    out=agg[:, it, :],
                                    in0=sc2,
                                    in1=agg[:, it, :],
                                    op=mybir.AluOpType.add,
                                )
                    # merge PSUM partial into SBUF agg; per-tile max
                    for k in range(ct):
                        it = c0 + k
                        nc.vector.tensor_add(
                            out=agg[:, it, :],
                            in0=agg[:, it, :],
                            in1=accs[k][:, 0:N],
                        )
                        nc.vector.tensor_reduce(
                            out=maxs[:, it : it + 1],
                            in_=agg[:, it, :],
                            axis=mybir.AxisListType.X,
                            op=mybir.AluOpType.max,
                        )

                # ---- softmax over the full [N, N] of this batch ----
                # cross-partition max: PE transpose -> free-axis reduce ->
                # broadcast back via a K=1 all-ones matmul.
                m1 = small_pool.tile([P, 1], f32, tag="m1")
                nc.vector.tensor_reduce(
                    out=m1, in_=maxs, axis=mybir.AxisListType.X,
                    op=mybir.AluOpType.max,
                )
                tps = acc_pool.tile([1, P], f32, tag="acc", name=f"tps_{b}")
                nc.tensor.transpose(tps, m1, ident_sb)
                gm = small_pool.tile([1, 1], f32, tag="gm")
                nc.vector.tensor_reduce(
                    out=gm, in_=tps, axis=mybir.AxisListType.X,
                    op=mybir.AluOpType.max,
                )
                bps = acc_pool.tile([P, 1], f32, tag="acc", name=f"bps_{b}")
                nc.tensor.matmul(bps, lhsT=ones_sb[0:1, :], rhs=gm,
                                 start=True, stop=True)
                negmax = small_pool.tile([P, 1], f32, tag="negmax")
                nc.scalar.mul(out=negmax, in_=bps, mul=-1.0)

                for it in range(NT):
                    nc.scalar.activation(
                        out=agg[:, it, :],
                        in_=agg[:, it, :],
                        func=mybir.ActivationFunctionType.Exp,
                        bias=negmax,
                        scale=1.0,
                        accum_out=sums[:, it : it + 1],
                    )
                s1 = small_pool.tile([P, 1], f32, tag="s1")
                nc.vector.tensor_reduce(
                    out=s1, in_=sums, axis=mybir.AxisListType.X,
                    op=mybir.AluOpType.add,
                )
                # cross-partition sum + broadcast in one all-ones matmul
                sps = acc_pool.tile([P, 1], f32, tag="acc", name=f"sps_{b}")
                nc.tensor.matmul(sps, lhsT=ones_sb, rhs=s1, start=True, stop=True)
                rinv = small_pool.tile([P, 1], f32, tag="rinv")
                nc.vector.reciprocal(out=rinv, in_=sps)

                for it in range(NT):
                    nc.vector.tensor_scalar_mul(
                        out=agg[:, it, :], in0=agg[:, it, :], scalar1=rinv
                    )
                dst = y[b].rearrange("(p t) j -> p t j", p=P)
                nc.sync.dma_start(out=dst, in_=agg)

    nc.finalize()  # Bacc: register alloc, nop/event-sem legalization, ISA codegen
    return nc


def kernel(mha_masks) -> np.ndarray:
    global LAST_RESULT
    from concourse.bass_utils import run_bass_kernel_spmd

    xfull = np.ascontiguousarray(np.asarray(mha_masks, dtype=np.float32))
    assert xfull.shape == (B, H, N, N), xfull.shape

    nc = build_program()
    ident = np.eye(P, dtype=np.float32)
    in_maps = [
        {"x": xfull[i * BPC : (i + 1) * BPC], "ident": ident}
        for i in range(NCORES)
    ]
    import os

    kw = {}
    if os.environ.get("KERNEL_TRACE_DIR"):
        kw = dict(trace=True, tmpdir=os.environ["KERNEL_TRACE_DIR"])
    res = run_bass_kernel_spmd(nc, in_maps, core_ids=list(range(NCORES)), **kw)
    LAST_RESULT = res
    out = np.concatenate(
        [np.asarray(r["y"], dtype=np.float32) for r in res.results], axis=0
    )
    return out
